# revision 17
# baseline (speedup 1.0000x reference)
"""Trainium2 Bass kernel for DifferentiableProjectionLayer (retrieval_knn).

Pipeline (per query point): KNN-8 over mesh vertices -> blended normal ->
ray cast (Moller-Trumbore) against mesh along -normal -> projected point.

Strategy:
- Host: bucket the 4096 queries into 32 buckets of 128 (4 equal-count theta
  bands x 8 equal-count phi buckets); per bucket select a candidate vertex
  window (KNN) and front-facing candidate face window (ray cast) with
  safety margins; build per-face linear coefficient tables.
- Device (SPMD, 8 cores x 4 buckets): per bucket
    negd2 = 2x.v - |v|^2 - |x|^2 via PE matmul (K=5)
    top-8 per query via DVE InstMax; masks via threshold compare
    weighted normal / nearest vertex via masked matmul over vertex chunks
    Moller-Trumbore: all validity conditions are LINEAR in the query
    features q = [x-c_b, n, (x-c_b) cross n, 1] (c_b = bucket center,
    n = blended unit normal, ray dir = -n). One matmul per condition
    block: g2,g3,g4 (pure sign tests) in fp16 with per-face positive
    scaling (sign-invariant; validated 0 changed rays on the target
    data); g5,g1 in fp32 (they also reconstruct t):
      valid = min(g1..g5) > 0,  D = g1 + 1e-9,  T = g5 + TOL*D,
      t = sum(valid*T)/sum(valid*D)   (convexity => unique front hit)
    xc = x - t*n; s = t*|n|^2.

The formulation was validated against the jax reference on the target
inputs (rel err ~2e-7, no decision flips that affect any ray).
"""

import numpy as np

P = 128
QB = 128           # queries per bucket
N_BANDS = 4
N_PHI = 8
N_CORES = 8
TOL = 1e-6
EPS = 1e-8
W_CONST = 0.01
VMARG_MULT = 1.6   # vertex window margin, units of max mesh edge chord
FMARG_MULT = 1.0   # face window margin

_B16 = ('g2', 'g3', 'g4')   # fp16 scaled condition blocks
_B32 = ('g5', 'g1')         # fp32 blocks (reconstruct t)
_compiled_cache = {}
LAST_RESULT = None  # BassKernelResults of the most recent run (for profiling)


# --------------------------------------------------------------------------
# host-side preparation
# --------------------------------------------------------------------------

def _angles(a):
    r = np.linalg.norm(a, axis=-1)
    th = np.arccos(np.clip(a[..., 2] / np.maximum(r, 1e-30), -1, 1))
    ph = np.arctan2(a[..., 1], a[..., 0])
    return th, ph


def _host_prep(x, vertices, vertex_normals, faces):
    N = x.shape[0]
    xf = x.astype(np.float64)
    vf = vertices.astype(np.float64)
    nf = vertex_normals.astype(np.float64)

    thq, phq = _angles(xf)
    thv, phv = _angles(vf)

    # --- equal-count buckets ---
    order_th = np.argsort(thq, kind='stable')
    per_band = N // N_BANDS
    buckets = []
    for b in range(N_BANDS):
        band_idx = order_th[b * per_band:(b + 1) * per_band]
        ph_ord = band_idx[np.argsort(phq[band_idx], kind='stable')]
        for j in range(per_band // QB):
            buckets.append(ph_ord[j * QB:(j + 1) * QB])
    perm = np.concatenate(buckets)

    # --- mesh scale ---
    tri = vf[faces]
    e_all = np.stack([tri[:, 1] - tri[:, 0], tri[:, 2] - tri[:, 0],
                      tri[:, 2] - tri[:, 1]], axis=1)
    delta = np.linalg.norm(e_all, axis=2).max()
    vmarg = VMARG_MULT * delta
    fmarg = FMARG_MULT * delta

    # --- face footprints ---
    tri_th = thv[faces]
    f_th_min = tri_th.min(axis=1)
    f_th_max = tri_th.max(axis=1)
    cs = np.cos(phv[faces]).sum(axis=1)
    sn = np.sin(phv[faces]).sum(axis=1)
    f_ph_c = np.arctan2(sn, cs)
    dd = np.abs(((phv[faces] - f_ph_c[:, None]) + np.pi) % (2 * np.pi) - np.pi)
    f_ph_hw = dd.max(axis=1)

    # --- canonicalize winding so n2 = e1 x e2 points outward ---
    v0 = tri[:, 0]
    e1 = tri[:, 1] - tri[:, 0]
    e2 = tri[:, 2] - tri[:, 0]
    n2 = np.cross(e1, e2)
    cent = tri.mean(axis=1)
    flip = (n2 * cent).sum(axis=1) < 0
    fcan = faces.copy()
    fcan[flip, 1], fcan[flip, 2] = faces[flip, 2], faces[flip, 1]
    tri = vf[fcan]
    v0 = tri[:, 0]
    e1 = tri[:, 1] - tri[:, 0]
    e2 = tri[:, 2] - tri[:, 0]
    n2 = np.cross(e1, e2)
    n2h = n2 / np.maximum(np.linalg.norm(n2, axis=1, keepdims=True), 1e-30)

    # --- per-face linear coefficients over [x(3), dirs(3), m=x X dirs(3), 1]
    F = faces.shape[0]
    c_e2v0 = np.cross(e2, v0)
    c_v0e1 = np.cross(v0, e1)
    c0 = (e2 * c_v0e1).sum(axis=1)

    D_ = np.zeros((F, 10)); D_[:, 3:6] = -n2
    U_ = np.zeros((F, 10)); U_[:, 6:9] = e2;  U_[:, 3:6] = -c_e2v0
    V_ = np.zeros((F, 10)); V_[:, 6:9] = -e1; V_[:, 3:6] = -c_v0e1
    T_ = np.zeros((F, 10)); T_[:, 0:3] = n2;  T_[:, 9] = -c0
    one = np.zeros((F, 10)); one[:, 9] = 1.0
    blocks = {
        'g1': D_ - 1e-9 * one,
        'g2': U_ + TOL * D_,
        'g3': V_ + TOL * D_,
        'g4': (1 + TOL) * D_ - U_ - V_,
        'g5': T_ - TOL * D_,
    }

    # --- per-bucket windows ---
    vert_wins, face_wins, centers = [], [], []
    for bidx in buckets:
        th_lo, th_hi = thq[bidx].min(), thq[bidx].max()
        ph = phq[bidx]
        ph_c = np.arctan2(np.sin(ph).sum(), np.cos(ph).sum())
        dd = ((ph - ph_c) + np.pi) % (2 * np.pi) - np.pi
        ph_lo, ph_hi = dd.min(), dd.max()

        th_lo_x = max(th_lo - fmarg, 1e-3)
        th_hi_x = min(th_hi + fmarg, np.pi - 1e-3)
        sin_min = min(np.sin(th_lo_x), np.sin(th_hi_x))

        vm_phi = vmarg / sin_min
        dv = ((phv - ph_c) + np.pi) % (2 * np.pi) - np.pi
        vok = ((thv >= th_lo - vmarg) & (thv <= th_hi + vmarg)
               & (dv >= ph_lo - vm_phi) & (dv <= ph_hi + vm_phi))
        vert_wins.append(np.nonzero(vok)[0])

        fm_phi = fmarg / sin_min
        df = ((f_ph_c - ph_c) + np.pi) % (2 * np.pi) - np.pi
        cdir = np.stack([np.sin(thq[bidx]) * np.cos(phq[bidx]),
                         np.sin(thq[bidx]) * np.sin(phq[bidx]),
                         np.cos(thq[bidx])], axis=1).mean(axis=0)
        cdir /= np.linalg.norm(cdir)
        fok = ((f_th_max >= th_lo - fmarg) & (f_th_min <= th_hi + fmarg)
               & (df + f_ph_hw >= ph_lo - fm_phi)
               & (df - f_ph_hw <= ph_hi + fm_phi)
               & (n2h @ cdir > 0.0))
        face_wins.append(np.nonzero(fok)[0])
        centers.append(xf[bidx].mean(axis=0))

    Vp = int(np.ceil(max(len(w) for w in vert_wins) / 128) * 128)
    Fp = int(np.ceil(max(len(w) for w in face_wins) / 128) * 128)
    Vp = max(Vp, 256)
    Fp = max(Fp, 128)
    VC = Vp // 128

    # --- per-bucket device tables ---
    # device feature basis: [xt = x - c_b, p = n = -dirs, mp = xt X p, 1]
    # original basis [x, dirs, m = x X dirs, 1]; with x = xt + cb,
    # dirs = -p:  m = -(xt X p) - cb X p  and  c_m.(cb X p) = p.(c_m X cb):
    #   row_xt = c_x; row_p = -c_d - c_m X cb; row_mp = -c_m;
    #   const += c_x . cb
    n_buckets = len(buckets)
    xq_all = np.zeros((n_buckets, QB, 3), np.float32)
    xqc_all = np.zeros((n_buckets, QB, 3), np.float32)
    vfeat_all = np.zeros((n_buckets, 5, Vp), np.float32)
    vtab_all = np.zeros((n_buckets, P, VC, 7), np.float32)
    wf16_all = np.zeros((n_buckets, 10, len(_B16), Fp), np.float16)
    wf32_all = np.zeros((n_buckets, 10, len(_B32), Fp), np.float32)

    for bi, bidx in enumerate(buckets):
        xq_all[bi] = xf[bidx].astype(np.float32)
        cb = centers[bi]
        xqc_all[bi] = (xf[bidx] - cb).astype(np.float32)

        vw = vert_wins[bi]
        nv = len(vw)
        vv = vf[vw]
        feat = np.zeros((5, Vp), np.float64)
        feat[3, :] = -1e30          # padding: negd2 = -1e30 (never selected)
        feat[4, :] = -1.0           # multiplies +|x|^2 from the query side
        feat[0:3, :nv] = 2.0 * vv.T
        feat[3, :nv] = -(vv * vv).sum(axis=1)
        vfeat_all[bi] = feat.astype(np.float32)

        vt = np.zeros((Vp, 7), np.float64)
        vt[:nv, 0:3] = -nf[vw]
        vt[:nv, 3] = -1.0
        vt[:nv, 4:7] = vv
        vtab_all[bi] = vt.reshape(VC, P, 7).transpose(1, 0, 2).astype(np.float32)

        fw = face_wins[bi]
        nfc = len(fw)

        def dev_basis(W):
            Wd = np.zeros_like(W)
            cxW = W[:, 0:3]
            cdW = W[:, 3:6]
            cmW = W[:, 6:9]
            Wd[:, 0:3] = cxW
            Wd[:, 3:6] = -cdW - np.cross(cmW, np.tile(cb, (W.shape[0], 1)))
            Wd[:, 6:9] = -cmW
            Wd[:, 9] = W[:, 9] + cxW @ cb
            return Wd

        for k, key in enumerate(_B16):
            Wd = dev_basis(blocks[key][fw])
            s = 1.0 / np.maximum(np.abs(Wd).max(axis=1), 1e-30)
            Wd = Wd * s[:, None]
            wf16_all[bi, :, k, :nfc] = Wd.T.astype(np.float16)
        for k, key in enumerate(_B32):
            Wd = dev_basis(blocks[key][fw])
            wf32_all[bi, :, k, :nfc] = Wd.T.astype(np.float32)

    return dict(perm=perm, xq=xq_all, xqc=xqc_all, vfeat=vfeat_all,
                vtab=vtab_all, wf16=wf16_all, wf32=wf32_all, Vp=Vp, Fp=Fp)


# --------------------------------------------------------------------------
# device kernel
# --------------------------------------------------------------------------

def _build_nc(Vp, Fp, NB):
    import concourse.bass as bass
    import concourse.mybir as mybir
    import concourse.tile as tile
    from concourse import bacc
    from concourse.masks import make_identity

    f32 = mybir.dt.float32
    f16 = mybir.dt.float16
    AF = mybir.ActivationFunctionType
    OP = mybir.AluOpType
    VC = Vp // 128
    NB16 = len(_B16)
    NB32 = len(_B32)
    assert Vp <= 512 and Fp <= 512

    nc = bacc.Bacc(None, target_bir_lowering=False)

    xq_d = nc.dram_tensor("xq", [NB, P, 3], f32, kind="ExternalInput")
    xqc_d = nc.dram_tensor("xqc", [NB, P, 3], f32, kind="ExternalInput")
    vfeat_d = nc.dram_tensor("vfeat", [NB, 5, Vp], f32, kind="ExternalInput")
    vtab_d = nc.dram_tensor("vtab", [NB, P, VC, 7], f32, kind="ExternalInput")
    wf16_d = nc.dram_tensor("wf16", [NB, 10, NB16, Fp], f16,
                            kind="ExternalInput")
    wf32_d = nc.dram_tensor("wf32", [NB, 10, NB32, Fp], f32,
                            kind="ExternalInput")
    out_d = nc.dram_tensor("out", [NB, P, 7], f32, kind="ExternalOutput")

    with tile.TileContext(nc) as tc:
        with (
            tc.tile_pool(name="persist", bufs=1) as persist,
            tc.tile_pool(name="sb", bufs=3) as sb,
            tc.tile_pool(name="sbB", bufs=3) as sbB,
            tc.tile_pool(name="junk", bufs=2) as junkp,
            tc.tile_pool(name="psmall", bufs=2, space="PSUM") as psmall,
            tc.tile_pool(name="ptr", bufs=2, space="PSUM") as ptr,
            tc.tile_pool(name="pg", bufs=2, space="PSUM") as pg,
        ):
            ident = persist.tile([P, P], f32)
            make_identity(nc, ident[:])

            def phase_a(b):
                st = {}
                # A = [x(3), 1, +|x|^2, pad(3)]  (query features for negd2)
                # B = [xt(3), p(3), mp(3), 1, pad(2)]  (ray-cast features)
                A = sb.tile([P, 8], f32, tag="A")
                B = st['B'] = sb.tile([P, 12], f32, name="B", tag="B")
                x_sb = st['x'] = sb.tile([P, 3], f32, name="x", tag="x")
                nc.sync.dma_start(x_sb[:], xq_d[b])
                nc.sync.dma_start(A[:, 0:3], xq_d[b])
                nc.sync.dma_start(B[:, 0:3], xqc_d[b])
                nc.gpsimd.memset(A[:, 3:4], 1.0)
                nc.gpsimd.memset(A[:, 5:8], 0.0)
                nc.gpsimd.memset(B[:, 9:10], 1.0)
                nc.gpsimd.memset(B[:, 10:12], 0.0)
                vf_sb = sb.tile([5, Vp], f32, tag="vfeat")
                nc.sync.dma_start(vf_sb[:], vfeat_d[b])
                vt_sb = sb.tile([P, VC, 7], f32, tag="vtab")
                nc.sync.dma_start(vt_sb[:], vtab_d[b])
                st['wf16'] = sb.tile([10, NB16, Fp], f16, name="wf16", tag="wf16")
                nc.sync.dma_start(st['wf16'][:], wf16_d[b])
                st['wf32'] = sb.tile([10, NB32, Fp], f32, name="wf32", tag="wf32")
                nc.sync.dma_start(st['wf32'][:], wf32_d[b])

                # query features (A[:,4] = +|x|^2)
                j3 = junkp.tile([P, 3], f32, tag="j3")
                nc.scalar.activation(j3[:], x_sb[:], AF.Square,
                                     accum_out=A[:, 4:5])
                at_ps = psmall.tile([8, P], f32, tag="small")
                nc.tensor.transpose(at_ps[:], A[:], ident[:])
                xfT = sb.tile([8, P], f32, tag="xfT")
                nc.scalar.copy(xfT[:], at_ps[:])

                # negd2 [128, Vp]
                nd_ps = psmall.tile([P, Vp], f32, tag="small")
                nc.tensor.matmul(nd_ps[:], xfT[0:5, :], vf_sb[:],
                                 start=True, stop=True)
                negd2 = sb.tile([P, Vp], f32, tag="negd2")
                nc.scalar.copy(negd2[:], nd_ps[:])

                neg8 = sb.tile([P, 8], f32, tag="neg8")
                nc.vector.max(neg8[:], negd2[:])

                # shifted copies (per-query thresholds live on partitions)
                sd = sb.tile([P, Vp], f32, tag="sd")
                nc.vector.tensor_scalar(sd[:], negd2[:], neg8[:, 7:8], None,
                                        op0=OP.subtract)
                sd1 = sb.tile([P, Vp], f32, tag="sd1")
                nc.vector.tensor_scalar(sd1[:], negd2[:], neg8[:, 0:1], None,
                                        op0=OP.subtract)

                # per-chunk: transpose to vertex-major, mask, accumulate
                trT = ptr.tile([P, 3, Vp], f32)
                rec = sb.tile([P, Vp], f32, tag="rec")
                wT = sb.tile([P, Vp], f32, tag="wT")
                w1T = sb.tile([P, Vp], f32, tag="w1T")
                term_ps = psmall.tile([P, 4], f32, tag="small")
                v1_ps = psmall.tile([P, 3], f32, tag="small")
                for c in range(VC):
                    cs = slice(c * P, (c + 1) * P)
                    nc.tensor.transpose(trT[:, 0, cs], negd2[:, cs], ident[:])
                    nc.tensor.transpose(trT[:, 1, cs], sd[:, cs], ident[:])
                    nc.tensor.transpose(trT[:, 2, cs], sd1[:, cs], ident[:])
                    nc.vector.reciprocal(rec[:, cs], trT[:, 0, cs])
                    nc.vector.scalar_tensor_tensor(wT[:, cs], trT[:, 1, cs],
                                                   0.0, rec[:, cs],
                                                   op0=OP.is_ge, op1=OP.mult)
                    nc.vector.tensor_scalar(w1T[:, cs], trT[:, 2, cs], 0.0,
                                            None, op0=OP.is_ge)
                    nc.tensor.matmul(term_ps[:], wT[:, cs], vt_sb[:, c, 0:4],
                                     start=(c == 0), stop=(c == VC - 1))
                    nc.tensor.matmul(v1_ps[:], w1T[:, cs], vt_sb[:, c, 4:7],
                                     start=(c == 0), stop=(c == VC - 1))
                term = sb.tile([P, 4], f32, tag="term")
                nc.scalar.copy(term[:], term_ps[:])
                v1 = sb.tile([P, 3], f32, tag="v1")
                nc.scalar.copy(v1[:], v1_ps[:])

                # blended normal direction u ~ wdir*term + (x - v1)
                wdir = sb.tile([P, 1], f32, tag="wdir")
                nc.vector.tensor_scalar(wdir[:], neg8[:, 0:1], -W_CONST,
                                        EPS * W_CONST, op0=OP.mult, op1=OP.max)
                diff = sb.tile([P, 3], f32, tag="diff")
                nc.vector.tensor_sub(diff[:], x_sb[:], v1[:])
                u = sb.tile([P, 3], f32, tag="u")
                nc.vector.scalar_tensor_tensor(u[:], term[:, 0:3], wdir[:],
                                               diff[:], op0=OP.mult,
                                               op1=OP.add)
                un2 = sb.tile([P, 1], f32, tag="un2")
                j3b = junkp.tile([P, 3], f32, tag="j3")
                nc.scalar.activation(j3b[:], u[:], AF.Square, accum_out=un2[:])
                unrm = sb.tile([P, 1], f32, tag="unrm")
                nc.scalar.activation(unrm[:], un2[:], AF.Sqrt)
                uninv = sb.tile([P, 1], f32, tag="uninv")
                nc.vector.reciprocal(uninv[:], unrm[:])

                # B: p = n = u/|u|; mp = xt cross p (strided column views)
                nc.vector.tensor_scalar(B[:, 3:6], u[:], uninv[:], None,
                                        op0=OP.mult)
                mtmp = sb.tile([P, 3], f32, tag="mtmp")
                nc.vector.tensor_mul(B[:, 6:8], B[:, 1:3], B[:, 5:2:-2])
                nc.vector.tensor_mul(mtmp[:, 0:2], B[:, 2::-2], B[:, 4:6])
                nc.vector.tensor_mul(B[:, 8:9], B[:, 0:1], B[:, 4:5])
                nc.vector.tensor_mul(mtmp[:, 2:3], B[:, 1:2], B[:, 3:4])
                nc.vector.tensor_sub(B[:, 6:9], B[:, 6:9], mtmp[:])

                nn2 = st['nn2'] = sb.tile([P, 1], f32, name="nn2", tag="nn2")
                j3c = junkp.tile([P, 3], f32, tag="j3")
                nc.scalar.activation(j3c[:], B[:, 3:6], AF.Square,
                                     accum_out=nn2[:])

                bt_ps = psmall.tile([12, P], f32, tag="small")
                nc.tensor.transpose(bt_ps[:], B[:], ident[:])
                qT = st['qT'] = sb.tile([10, P], f32, name="qT", tag="qT")
                nc.scalar.copy(qT[:], bt_ps[0:10, :])
                q16T = st['q16T'] = sb.tile([10, P], f16, name="q16T", tag="q16T")
                nc.scalar.copy(q16T[:], bt_ps[0:10, :])
                return st

            def phase_b(b, st):
                qT, q16T = st['qT'], st['q16T']
                wf16_sb, wf32_sb = st['wf16'], st['wf32']
                B, x_sb, nn2 = st['B'], st['x'], st['nn2']
                # Moller-Trumbore: g2,g3,g4 fp16; g5,g1 fp32
                g2 = pg.tile([P, Fp], f32, tag="g")
                nc.tensor.matmul(g2[:], q16T[:], wf16_sb[:, 0, :],
                                 start=True, stop=True)
                g3 = pg.tile([P, Fp], f32, tag="g")
                nc.tensor.matmul(g3[:], q16T[:], wf16_sb[:, 1, :],
                                 start=True, stop=True)
                g3s = sbB.tile([P, Fp], f32, tag="gs")
                nc.scalar.copy(g3s[:], g3[:])
                m1 = sbB.tile([P, Fp], f32, tag="m1")
                nc.vector.tensor_tensor(m1[:], g2[:], g3s[:], op=OP.min)

                g4 = pg.tile([P, Fp], f32, tag="g")
                nc.tensor.matmul(g4[:], q16T[:], wf16_sb[:, 2, :],
                                 start=True, stop=True)
                g5 = pg.tile([P, Fp], f32, tag="g")
                nc.tensor.matmul(g5[:], qT[:], wf32_sb[:, 0, :],
                                 start=True, stop=True)
                g5s = sbB.tile([P, Fp], f32, tag="gs")
                nc.scalar.copy(g5s[:], g5[:])
                m2 = sbB.tile([P, Fp], f32, tag="m2")
                nc.vector.tensor_tensor(m2[:], g4[:], g5s[:], op=OP.min)

                m3 = sbB.tile([P, Fp], f32, tag="m1")
                nc.vector.tensor_tensor(m3[:], m1[:], m2[:], op=OP.min)
                g1 = pg.tile([P, Fp], f32, tag="g")
                nc.tensor.matmul(g1[:], qT[:], wf32_sb[:, 1, :],
                                 start=True, stop=True)
                qmin = sbB.tile([P, Fp], f32, tag="m2")
                nc.vector.tensor_tensor(qmin[:], m3[:], g1[:], op=OP.min)

                # valid sums
                s1 = sb.tile([P, 1], f32, tag="s1")
                jA = junkp.tile([P, Fp], f32, tag="jF")
                nc.vector.scalar_tensor_tensor(jA[:], qmin[:], 0.0, g1[:],
                                               op0=OP.is_gt, op1=OP.mult,
                                               accum_out=s1[:])
                s5 = sb.tile([P, 1], f32, tag="s5")
                jB = junkp.tile([P, Fp], f32, tag="jF")
                nc.vector.scalar_tensor_tensor(jB[:], qmin[:], 0.0, g5s[:],
                                               op0=OP.is_gt, op1=OP.mult,
                                               accum_out=s5[:])
                # t and outputs: sD = s1, T = g5 + TOL*D
                sT = sb.tile([P, 1], f32, tag="sT")
                nc.vector.scalar_tensor_tensor(sT[:], s1[:], TOL, s5[:],
                                               op0=OP.mult, op1=OP.add)
                sDg = sb.tile([P, 1], f32, tag="sDg")
                nc.vector.tensor_scalar(sDg[:], s1[:], 1e-30, None, op0=OP.max)
                tden = sb.tile([P, 1], f32, tag="tden")
                nc.vector.reciprocal(tden[:], sDg[:])
                tneg = sb.tile([P, 1], f32, tag="tneg")
                nc.vector.scalar_tensor_tensor(tneg[:], sT[:], -1.0, tden[:],
                                               op0=OP.mult, op1=OP.mult)

                outt = sb.tile([P, 7], f32, tag="outt")
                # xc = x + t*dirs = x - t*p
                nc.vector.scalar_tensor_tensor(outt[:, 0:3], B[:, 3:6],
                                               tneg[:], x_sb[:],
                                               op0=OP.mult, op1=OP.add)
                nc.vector.scalar_tensor_tensor(outt[:, 3:4], tneg[:], -1.0,
                                               nn2[:], op0=OP.mult,
                                               op1=OP.mult)
                nc.scalar.copy(outt[:, 4:7], B[:, 3:6])
                nc.sync.dma_start(out_d[b], outt[:])

            # software pipeline: phase A of bucket b+1 overlaps phase B of b
            prev = phase_a(0)
            for b in range(1, NB):
                cur = phase_a(b)
                phase_b(b - 1, prev)
                prev = cur
            phase_b(NB - 1, prev)

    nc.compile()
    return nc


# --------------------------------------------------------------------------
# entry point
# --------------------------------------------------------------------------

def kernel(x, vertices, vertex_normals, faces):
    from concourse.bass_utils import run_bass_kernel_spmd

    x = np.asarray(x, np.float32)
    vertices = np.asarray(vertices, np.float32)
    vertex_normals = np.asarray(vertex_normals, np.float32)
    faces = np.asarray(faces, np.int32)

    prep = _host_prep(x, vertices, vertex_normals, faces)
    Vp, Fp = prep['Vp'], prep['Fp']
    n_buckets = prep['xq'].shape[0]
    NB = n_buckets // N_CORES

    key = (Vp, Fp, NB)
    if key not in _compiled_cache:
        _compiled_cache[key] = _build_nc(Vp, Fp, NB)
    nc = _compiled_cache[key]

    in_maps = []
    for c in range(N_CORES):
        sl = slice(c * NB, (c + 1) * NB)
        in_maps.append({
            'xq': np.ascontiguousarray(prep['xq'][sl]),
            'xqc': np.ascontiguousarray(prep['xqc'][sl]),
            'vfeat': np.ascontiguousarray(prep['vfeat'][sl]),
            'vtab': np.ascontiguousarray(prep['vtab'][sl]),
            'wf16': np.ascontiguousarray(prep['wf16'][sl]),
            'wf32': np.ascontiguousarray(prep['wf32'][sl]),
        })

    global LAST_RESULT
    outs = None
    for attempt in range(3):
        res = run_bass_kernel_spmd(nc, in_maps, core_ids=list(range(N_CORES)))
        LAST_RESULT = res
        outs = np.concatenate([r['out'].reshape(NB * P, 7)
                               for r in res.results], axis=0)
        # outputs are geometrically bounded (|xc|<~3, |s|<~1, |n|=1); a
        # wildly out-of-range or non-finite result indicates a transient
        # device-state failure -> rerun
        if np.isfinite(outs).all() and np.abs(outs).max() < 1e3:
            break
    assert outs is not None

    N = x.shape[0]
    full = np.zeros((N, 7), np.float32)
    full[prep['perm']] = outs
    xc = np.ascontiguousarray(full[:, 0:3])
    s = np.ascontiguousarray(full[:, 3:4])
    n = np.ascontiguousarray(full[:, 4:7])
    return xc, s, n


# revision 18
# speedup vs baseline: 1.1281x; 1.1281x over previous
"""Trainium2 Bass kernel for DifferentiableProjectionLayer (retrieval_knn).

Pipeline (per query point): KNN-8 over mesh vertices -> blended normal ->
ray cast (Moller-Trumbore) against mesh along -normal -> projected point.

Strategy:
- Host: bucket the 4096 queries into 32 buckets of 128 (4 equal-count theta
  bands x 8 equal-count phi buckets); per bucket select a candidate vertex
  window (KNN) and front-facing candidate face window (ray cast) with
  safety margins; build per-face linear coefficient tables.
- Device (SPMD, 8 cores x 4 buckets): per bucket
    negd2 = 2x.v - |v|^2 - |x|^2 via PE matmul (K=5)
    top-8 per query via DVE InstMax; masks via threshold compare
    weighted normal / nearest vertex via masked matmul over vertex chunks
    Moller-Trumbore: all validity conditions are LINEAR in the query
    features q = [x-c_b, n, (x-c_b) cross n, 1] (c_b = bucket center,
    n = blended unit normal, ray dir = -n). One matmul per condition
    block: g2,g3,g4 (pure sign tests) in fp16 with per-face positive
    scaling (sign-invariant; validated 0 changed rays on the target
    data); g5,g1 in fp32 (they also reconstruct t):
      valid = min(g1..g5) > 0,  D = g1 + 1e-9,  T = g5 + TOL*D,
      t = sum(valid*T)/sum(valid*D)   (convexity => unique front hit)
    xc = x - t*n; s = t*|n|^2.

The formulation was validated against the jax reference on the target
inputs (rel err ~2e-7, no decision flips that affect any ray).
"""

import numpy as np

P = 128
QB = 128           # queries per bucket
N_BANDS = 4
N_PHI = 8
N_CORES = 8
TOL = 1e-6
EPS = 1e-8
W_CONST = 0.01
VMARG_MULT = 1.6   # vertex window margin, units of max mesh edge chord
FMARG_MULT = 1.0   # face window margin

_B16 = ('g2', 'g3', 'g4')   # fp16 scaled condition blocks
_B32 = ('g5', 'g1')         # fp32 blocks (reconstruct t)
_compiled_cache = {}
LAST_RESULT = None  # BassKernelResults of the most recent run (for profiling)


# --------------------------------------------------------------------------
# host-side preparation
# --------------------------------------------------------------------------

def _angles(a):
    r = np.linalg.norm(a, axis=-1)
    th = np.arccos(np.clip(a[..., 2] / np.maximum(r, 1e-30), -1, 1))
    ph = np.arctan2(a[..., 1], a[..., 0])
    return th, ph


def _host_prep(x, vertices, vertex_normals, faces):
    N = x.shape[0]
    xf = x.astype(np.float64)
    vf = vertices.astype(np.float64)
    nf = vertex_normals.astype(np.float64)

    thq, phq = _angles(xf)
    thv, phv = _angles(vf)

    # --- equal-count buckets ---
    order_th = np.argsort(thq, kind='stable')
    per_band = N // N_BANDS
    buckets = []
    for b in range(N_BANDS):
        band_idx = order_th[b * per_band:(b + 1) * per_band]
        ph_ord = band_idx[np.argsort(phq[band_idx], kind='stable')]
        for j in range(per_band // QB):
            buckets.append(ph_ord[j * QB:(j + 1) * QB])
    perm = np.concatenate(buckets)

    # --- mesh scale ---
    tri = vf[faces]
    e_all = np.stack([tri[:, 1] - tri[:, 0], tri[:, 2] - tri[:, 0],
                      tri[:, 2] - tri[:, 1]], axis=1)
    delta = np.linalg.norm(e_all, axis=2).max()
    vmarg = VMARG_MULT * delta
    fmarg = FMARG_MULT * delta

    # --- face footprints ---
    tri_th = thv[faces]
    f_th_min = tri_th.min(axis=1)
    f_th_max = tri_th.max(axis=1)
    cs = np.cos(phv[faces]).sum(axis=1)
    sn = np.sin(phv[faces]).sum(axis=1)
    f_ph_c = np.arctan2(sn, cs)
    dd = np.abs(((phv[faces] - f_ph_c[:, None]) + np.pi) % (2 * np.pi) - np.pi)
    f_ph_hw = dd.max(axis=1)

    # --- canonicalize winding so n2 = e1 x e2 points outward ---
    v0 = tri[:, 0]
    e1 = tri[:, 1] - tri[:, 0]
    e2 = tri[:, 2] - tri[:, 0]
    n2 = np.cross(e1, e2)
    cent = tri.mean(axis=1)
    flip = (n2 * cent).sum(axis=1) < 0
    fcan = faces.copy()
    fcan[flip, 1], fcan[flip, 2] = faces[flip, 2], faces[flip, 1]
    tri = vf[fcan]
    v0 = tri[:, 0]
    e1 = tri[:, 1] - tri[:, 0]
    e2 = tri[:, 2] - tri[:, 0]
    n2 = np.cross(e1, e2)
    n2h = n2 / np.maximum(np.linalg.norm(n2, axis=1, keepdims=True), 1e-30)

    # --- per-face linear coefficients over [x(3), dirs(3), m=x X dirs(3), 1]
    F = faces.shape[0]
    c_e2v0 = np.cross(e2, v0)
    c_v0e1 = np.cross(v0, e1)
    c0 = (e2 * c_v0e1).sum(axis=1)

    D_ = np.zeros((F, 10)); D_[:, 3:6] = -n2
    U_ = np.zeros((F, 10)); U_[:, 6:9] = e2;  U_[:, 3:6] = -c_e2v0
    V_ = np.zeros((F, 10)); V_[:, 6:9] = -e1; V_[:, 3:6] = -c_v0e1
    T_ = np.zeros((F, 10)); T_[:, 0:3] = n2;  T_[:, 9] = -c0
    one = np.zeros((F, 10)); one[:, 9] = 1.0
    blocks = {
        'g1': D_ - 1e-9 * one,
        'g2': U_ + TOL * D_,
        'g3': V_ + TOL * D_,
        'g4': (1 + TOL) * D_ - U_ - V_,
        'g5': T_ - TOL * D_,
    }

    # --- per-bucket windows ---
    vert_wins, face_wins, centers = [], [], []
    for bidx in buckets:
        th_lo, th_hi = thq[bidx].min(), thq[bidx].max()
        ph = phq[bidx]
        ph_c = np.arctan2(np.sin(ph).sum(), np.cos(ph).sum())
        dd = ((ph - ph_c) + np.pi) % (2 * np.pi) - np.pi
        ph_lo, ph_hi = dd.min(), dd.max()

        th_lo_x = max(th_lo - fmarg, 1e-3)
        th_hi_x = min(th_hi + fmarg, np.pi - 1e-3)
        sin_min = min(np.sin(th_lo_x), np.sin(th_hi_x))

        vm_phi = vmarg / sin_min
        dv = ((phv - ph_c) + np.pi) % (2 * np.pi) - np.pi
        vok = ((thv >= th_lo - vmarg) & (thv <= th_hi + vmarg)
               & (dv >= ph_lo - vm_phi) & (dv <= ph_hi + vm_phi))
        vert_wins.append(np.nonzero(vok)[0])

        fm_phi = fmarg / sin_min
        df = ((f_ph_c - ph_c) + np.pi) % (2 * np.pi) - np.pi
        cdir = np.stack([np.sin(thq[bidx]) * np.cos(phq[bidx]),
                         np.sin(thq[bidx]) * np.sin(phq[bidx]),
                         np.cos(thq[bidx])], axis=1).mean(axis=0)
        cdir /= np.linalg.norm(cdir)
        fok = ((f_th_max >= th_lo - fmarg) & (f_th_min <= th_hi + fmarg)
               & (df + f_ph_hw >= ph_lo - fm_phi)
               & (df - f_ph_hw <= ph_hi + fm_phi)
               & (n2h @ cdir > 0.0))
        face_wins.append(np.nonzero(fok)[0])
        centers.append(xf[bidx].mean(axis=0))

    Vp = int(np.ceil(max(len(w) for w in vert_wins) / 128) * 128)
    Fp = int(np.ceil(max(len(w) for w in face_wins) / 128) * 128)
    Vp = max(Vp, 256)
    Fp = max(Fp, 128)
    VC = Vp // 128

    # --- per-bucket device tables ---
    # device feature basis: [xt = x - c_b, p = n = -dirs, mp = xt X p, 1]
    # original basis [x, dirs, m = x X dirs, 1]; with x = xt + cb,
    # dirs = -p:  m = -(xt X p) - cb X p  and  c_m.(cb X p) = p.(c_m X cb):
    #   row_xt = c_x; row_p = -c_d - c_m X cb; row_mp = -c_m;
    #   const += c_x . cb
    n_buckets = len(buckets)
    xq_all = np.zeros((n_buckets, QB, 3), np.float32)
    xqc_all = np.zeros((n_buckets, QB, 3), np.float32)
    vfeat_all = np.zeros((n_buckets, 5, Vp), np.float32)
    vtab_all = np.zeros((n_buckets, P, VC, 7), np.float32)
    import ml_dtypes
    bf16 = ml_dtypes.bfloat16
    wf16_all = np.zeros((n_buckets, 10, len(_B16), Fp), np.float16)
    wfhl_all = np.zeros((n_buckets, 10, len(_B32), 2, Fp), bf16)

    for bi, bidx in enumerate(buckets):
        xq_all[bi] = xf[bidx].astype(np.float32)
        cb = centers[bi]
        xqc_all[bi] = (xf[bidx] - cb).astype(np.float32)

        vw = vert_wins[bi]
        nv = len(vw)
        vv = vf[vw]
        feat = np.zeros((5, Vp), np.float64)
        feat[3, :] = -1e30          # padding: negd2 = -1e30 (never selected)
        feat[4, :] = -1.0           # multiplies +|x|^2 from the query side
        feat[0:3, :nv] = 2.0 * vv.T
        feat[3, :nv] = -(vv * vv).sum(axis=1)
        vfeat_all[bi] = feat.astype(np.float32)

        vt = np.zeros((Vp, 7), np.float64)
        vt[:nv, 0:3] = -nf[vw]
        vt[:nv, 3] = -1.0
        vt[:nv, 4:7] = vv
        vtab_all[bi] = vt.reshape(VC, P, 7).transpose(1, 0, 2).astype(np.float32)

        fw = face_wins[bi]
        nfc = len(fw)

        def dev_basis(W):
            Wd = np.zeros_like(W)
            cxW = W[:, 0:3]
            cdW = W[:, 3:6]
            cmW = W[:, 6:9]
            Wd[:, 0:3] = cxW
            Wd[:, 3:6] = -cdW - np.cross(cmW, np.tile(cb, (W.shape[0], 1)))
            Wd[:, 6:9] = -cmW
            Wd[:, 9] = W[:, 9] + cxW @ cb
            return Wd

        for k, key in enumerate(_B16):
            Wd = dev_basis(blocks[key][fw])
            s = 1.0 / np.maximum(np.abs(Wd).max(axis=1), 1e-30)
            Wd = Wd * s[:, None]
            wf16_all[bi, :, k, :nfc] = Wd.T.astype(np.float16)
        for k, key in enumerate(_B32):
            Wd = dev_basis(blocks[key][fw]).astype(np.float32)
            Wh = Wd.astype(bf16).astype(np.float32)
            Wl = (Wd - Wh).astype(bf16)
            wfhl_all[bi, :, k, 0, :nfc] = Wh.T.astype(bf16)
            wfhl_all[bi, :, k, 1, :nfc] = Wl.T

    return dict(perm=perm, xq=xq_all, xqc=xqc_all, vfeat=vfeat_all,
                vtab=vtab_all, wf16=wf16_all, wfhl=wfhl_all, Vp=Vp, Fp=Fp)


# --------------------------------------------------------------------------
# device kernel
# --------------------------------------------------------------------------

def _build_nc(Vp, Fp, NB):
    import concourse.bass as bass
    import concourse.mybir as mybir
    import concourse.tile as tile
    from concourse import bacc
    from concourse.masks import make_identity

    f32 = mybir.dt.float32
    f16 = mybir.dt.float16
    bf16 = mybir.dt.bfloat16
    AF = mybir.ActivationFunctionType
    OP = mybir.AluOpType
    VC = Vp // 128
    NB16 = len(_B16)
    NB32 = len(_B32)
    assert Vp <= 512 and Fp <= 512

    nc = bacc.Bacc(None, target_bir_lowering=False)

    xq_d = nc.dram_tensor("xq", [NB, P, 3], f32, kind="ExternalInput")
    xqc_d = nc.dram_tensor("xqc", [NB, P, 3], f32, kind="ExternalInput")
    vfeat_d = nc.dram_tensor("vfeat", [NB, 5, Vp], f32, kind="ExternalInput")
    vtab_d = nc.dram_tensor("vtab", [NB, P, VC, 7], f32, kind="ExternalInput")
    wf16_d = nc.dram_tensor("wf16", [NB, 10, NB16, Fp], f16,
                            kind="ExternalInput")
    wfhl_d = nc.dram_tensor("wfhl", [NB, 10, NB32, 2, Fp], bf16,
                            kind="ExternalInput")
    out_d = nc.dram_tensor("out", [NB, P, 7], f32, kind="ExternalOutput")

    with tile.TileContext(nc) as tc:
        with (
            tc.tile_pool(name="persist", bufs=1) as persist,
            tc.tile_pool(name="sb", bufs=3) as sb,
            tc.tile_pool(name="sbB", bufs=3) as sbB,
            tc.tile_pool(name="junk", bufs=2) as junkp,
            tc.tile_pool(name="psmall", bufs=2, space="PSUM") as psmall,
            tc.tile_pool(name="ptr", bufs=2, space="PSUM") as ptr,
            tc.tile_pool(name="pg", bufs=2, space="PSUM") as pg,
        ):
            ident = persist.tile([P, P], f32)
            make_identity(nc, ident[:])

            def phase_a(b):
                st = {}
                # A = [x(3), 1, +|x|^2, pad(3)]  (query features for negd2)
                # B = [xt(3), p(3), mp(3), 1, pad(2)]  (ray-cast features)
                A = sb.tile([P, 8], f32, tag="A")
                B = st['B'] = sb.tile([P, 12], f32, name="B", tag="B")
                x_sb = st['x'] = sb.tile([P, 3], f32, name="x", tag="x")
                nc.sync.dma_start(x_sb[:], xq_d[b])
                nc.sync.dma_start(A[:, 0:3], xq_d[b])
                nc.sync.dma_start(B[:, 0:3], xqc_d[b])
                nc.gpsimd.memset(A[:, 3:4], 1.0)
                nc.gpsimd.memset(A[:, 5:8], 0.0)
                nc.gpsimd.memset(B[:, 9:10], 1.0)
                nc.gpsimd.memset(B[:, 10:12], 0.0)
                vf_sb = sb.tile([5, Vp], f32, tag="vfeat")
                nc.sync.dma_start(vf_sb[:], vfeat_d[b])
                vt_sb = sb.tile([P, VC, 7], f32, tag="vtab")
                nc.sync.dma_start(vt_sb[:], vtab_d[b])
                st['wf16'] = sb.tile([10, NB16, Fp], f16, name="wf16", tag="wf16")
                nc.sync.dma_start(st['wf16'][:], wf16_d[b])
                st['wfhl'] = sb.tile([10, NB32, 2, Fp], bf16, name="wfhl",
                                     tag="wfhl")
                nc.sync.dma_start(st['wfhl'][:], wfhl_d[b])

                # query features (A[:,4] = +|x|^2)
                j3 = junkp.tile([P, 3], f32, tag="j3")
                nc.scalar.activation(j3[:], x_sb[:], AF.Square,
                                     accum_out=A[:, 4:5])
                at_ps = psmall.tile([8, P], f32, tag="small")
                nc.tensor.transpose(at_ps[:], A[:], ident[:])
                xfT = sb.tile([8, P], f32, tag="xfT")
                nc.scalar.copy(xfT[:], at_ps[:])

                # negd2 [128, Vp]
                nd_ps = psmall.tile([P, Vp], f32, tag="small")
                nc.tensor.matmul(nd_ps[:], xfT[0:5, :], vf_sb[:],
                                 start=True, stop=True)
                negd2 = sb.tile([P, Vp], f32, tag="negd2")
                nc.scalar.copy(negd2[:], nd_ps[:])

                neg8 = sb.tile([P, 8], f32, tag="neg8")
                nc.vector.max(neg8[:], negd2[:])

                # shifted copies (per-query thresholds live on partitions)
                sd = sb.tile([P, Vp], f32, tag="sd")
                nc.vector.tensor_scalar(sd[:], negd2[:], neg8[:, 7:8], None,
                                        op0=OP.subtract)
                sd1 = sb.tile([P, Vp], f32, tag="sd1")
                nc.vector.tensor_scalar(sd1[:], negd2[:], neg8[:, 0:1], None,
                                        op0=OP.subtract)

                # per-chunk: transpose to vertex-major, mask, accumulate
                trT = ptr.tile([P, 3, Vp], f32)
                rec = sb.tile([P, Vp], f32, tag="rec")
                wT = sb.tile([P, Vp], f32, tag="wT")
                w1T = sb.tile([P, Vp], f32, tag="w1T")
                term_ps = psmall.tile([P, 4], f32, tag="small")
                v1_ps = psmall.tile([P, 3], f32, tag="small")
                for c in range(VC):
                    cs = slice(c * P, (c + 1) * P)
                    nc.tensor.transpose(trT[:, 0, cs], negd2[:, cs], ident[:])
                    nc.tensor.transpose(trT[:, 1, cs], sd[:, cs], ident[:])
                    nc.tensor.transpose(trT[:, 2, cs], sd1[:, cs], ident[:])
                    nc.vector.reciprocal_approx_fast(rec[:, cs], trT[:, 0, cs])
                    nc.vector.scalar_tensor_tensor(wT[:, cs], trT[:, 1, cs],
                                                   0.0, rec[:, cs],
                                                   op0=OP.is_ge, op1=OP.mult)
                    nc.vector.tensor_scalar(w1T[:, cs], trT[:, 2, cs], 0.0,
                                            None, op0=OP.is_ge)
                    nc.tensor.matmul(term_ps[:], wT[:, cs], vt_sb[:, c, 0:4],
                                     start=(c == 0), stop=(c == VC - 1))
                    nc.tensor.matmul(v1_ps[:], w1T[:, cs], vt_sb[:, c, 4:7],
                                     start=(c == 0), stop=(c == VC - 1))
                term = sb.tile([P, 4], f32, tag="term")
                nc.scalar.copy(term[:], term_ps[:])
                v1 = sb.tile([P, 3], f32, tag="v1")
                nc.scalar.copy(v1[:], v1_ps[:])

                # blended normal direction u ~ wdir*term + (x - v1)
                wdir = sb.tile([P, 1], f32, tag="wdir")
                nc.vector.tensor_scalar(wdir[:], neg8[:, 0:1], -W_CONST,
                                        EPS * W_CONST, op0=OP.mult, op1=OP.max)
                diff = sb.tile([P, 3], f32, tag="diff")
                nc.vector.tensor_sub(diff[:], x_sb[:], v1[:])
                u = sb.tile([P, 3], f32, tag="u")
                nc.vector.scalar_tensor_tensor(u[:], term[:, 0:3], wdir[:],
                                               diff[:], op0=OP.mult,
                                               op1=OP.add)
                un2 = sb.tile([P, 1], f32, tag="un2")
                j3b = junkp.tile([P, 3], f32, tag="j3")
                nc.scalar.activation(j3b[:], u[:], AF.Square, accum_out=un2[:])
                unrm = sb.tile([P, 1], f32, tag="unrm")
                nc.scalar.activation(unrm[:], un2[:], AF.Sqrt)
                uninv = sb.tile([P, 1], f32, tag="uninv")
                nc.vector.reciprocal_approx_fast(uninv[:], unrm[:])

                # B: p = n = u/|u|; mp = xt cross p (strided column views)
                nc.vector.tensor_scalar(B[:, 3:6], u[:], uninv[:], None,
                                        op0=OP.mult)
                mtmp = sb.tile([P, 3], f32, tag="mtmp")
                nc.vector.tensor_mul(B[:, 6:8], B[:, 1:3], B[:, 5:2:-2])
                nc.vector.tensor_mul(mtmp[:, 0:2], B[:, 2::-2], B[:, 4:6])
                nc.vector.tensor_mul(B[:, 8:9], B[:, 0:1], B[:, 4:5])
                nc.vector.tensor_mul(mtmp[:, 2:3], B[:, 1:2], B[:, 3:4])
                nc.vector.tensor_sub(B[:, 6:9], B[:, 6:9], mtmp[:])

                nn2 = st['nn2'] = sb.tile([P, 1], f32, name="nn2", tag="nn2")
                j3c = junkp.tile([P, 3], f32, tag="j3")
                nc.scalar.activation(j3c[:], B[:, 3:6], AF.Square,
                                     accum_out=nn2[:])

                bt_ps = psmall.tile([12, P], f32, tag="small")
                nc.tensor.transpose(bt_ps[:], B[:], ident[:])
                q16T = st['q16T'] = sb.tile([10, P], f16, name="q16T", tag="q16T")
                nc.scalar.copy(q16T[:], bt_ps[0:10, :])
                qh = st['qh'] = sb.tile([10, P], bf16, name="qh", tag="qh")
                nc.scalar.copy(qh[:], bt_ps[0:10, :])
                qhf = sb.tile([10, P], f32, name="qhf", tag="qhf")
                nc.scalar.copy(qhf[:], qh[:])
                qlf = sb.tile([10, P], f32, name="qlf", tag="qlf")
                nc.vector.tensor_sub(qlf[:], bt_ps[0:10, :], qhf[:])
                ql = st['ql'] = sb.tile([10, P], bf16, name="ql", tag="ql")
                nc.scalar.copy(ql[:], qlf[:])
                return st

            def phase_b(b, st):
                q16T, qh, ql = st['q16T'], st['qh'], st['ql']
                wf16_sb, wfhl_sb = st['wf16'], st['wfhl']
                B, x_sb, nn2 = st['B'], st['x'], st['nn2']

                def mm_split(out, k):
                    nc.tensor.matmul(out, qh[:], wfhl_sb[:, k, 0, :],
                                     start=True, stop=False)
                    nc.tensor.matmul(out, ql[:], wfhl_sb[:, k, 0, :],
                                     start=False, stop=False)
                    nc.tensor.matmul(out, qh[:], wfhl_sb[:, k, 1, :],
                                     start=False, stop=True)
                # Moller-Trumbore: g2,g3,g4 fp16; g5,g1 fp32
                g2 = pg.tile([P, Fp], f32, tag="g")
                nc.tensor.matmul(g2[:], q16T[:], wf16_sb[:, 0, :],
                                 start=True, stop=True)
                g3 = pg.tile([P, Fp], f32, tag="g")
                nc.tensor.matmul(g3[:], q16T[:], wf16_sb[:, 1, :],
                                 start=True, stop=True)
                g3s = sbB.tile([P, Fp], f32, tag="gs")
                nc.scalar.copy(g3s[:], g3[:])
                m1 = sbB.tile([P, Fp], f32, tag="m1")
                nc.vector.tensor_tensor(m1[:], g2[:], g3s[:], op=OP.min)

                g4 = pg.tile([P, Fp], f32, tag="g")
                nc.tensor.matmul(g4[:], q16T[:], wf16_sb[:, 2, :],
                                 start=True, stop=True)
                g5 = pg.tile([P, Fp], f32, tag="g")
                mm_split(g5[:], 0)
                g5s = sbB.tile([P, Fp], f32, tag="gs")
                nc.scalar.copy(g5s[:], g5[:])
                m2 = sbB.tile([P, Fp], f32, tag="m2")
                nc.vector.tensor_tensor(m2[:], g4[:], g5s[:], op=OP.min)

                m3 = sbB.tile([P, Fp], f32, tag="m1")
                nc.vector.tensor_tensor(m3[:], m1[:], m2[:], op=OP.min)
                g1 = pg.tile([P, Fp], f32, tag="g")
                mm_split(g1[:], 1)
                qmin = sbB.tile([P, Fp], f32, tag="m2")
                nc.vector.tensor_tensor(qmin[:], m3[:], g1[:], op=OP.min)

                # valid sums
                s1 = sb.tile([P, 1], f32, tag="s1")
                jA = junkp.tile([P, Fp], f32, tag="jF")
                nc.vector.scalar_tensor_tensor(jA[:], qmin[:], 0.0, g1[:],
                                               op0=OP.is_gt, op1=OP.mult,
                                               accum_out=s1[:])
                s5 = sb.tile([P, 1], f32, tag="s5")
                jB = junkp.tile([P, Fp], f32, tag="jF")
                nc.vector.scalar_tensor_tensor(jB[:], qmin[:], 0.0, g5s[:],
                                               op0=OP.is_gt, op1=OP.mult,
                                               accum_out=s5[:])
                # t and outputs: sD = s1, T = g5 + TOL*D
                sT = sb.tile([P, 1], f32, tag="sT")
                nc.vector.scalar_tensor_tensor(sT[:], s1[:], TOL, s5[:],
                                               op0=OP.mult, op1=OP.add)
                sDg = sb.tile([P, 1], f32, tag="sDg")
                nc.vector.tensor_scalar(sDg[:], s1[:], 1e-30, None, op0=OP.max)
                tden = sb.tile([P, 1], f32, tag="tden")
                nc.vector.reciprocal_approx_fast(tden[:], sDg[:])
                tneg = sb.tile([P, 1], f32, tag="tneg")
                nc.vector.scalar_tensor_tensor(tneg[:], sT[:], -1.0, tden[:],
                                               op0=OP.mult, op1=OP.mult)

                outt = sb.tile([P, 7], f32, tag="outt")
                # xc = x + t*dirs = x - t*p
                nc.vector.scalar_tensor_tensor(outt[:, 0:3], B[:, 3:6],
                                               tneg[:], x_sb[:],
                                               op0=OP.mult, op1=OP.add)
                nc.vector.scalar_tensor_tensor(outt[:, 3:4], tneg[:], -1.0,
                                               nn2[:], op0=OP.mult,
                                               op1=OP.mult)
                nc.scalar.copy(outt[:, 4:7], B[:, 3:6])
                nc.sync.dma_start(out_d[b], outt[:])

            # software pipeline: phase A of bucket b+1 overlaps phase B of b
            prev = phase_a(0)
            for b in range(1, NB):
                cur = phase_a(b)
                phase_b(b - 1, prev)
                prev = cur
            phase_b(NB - 1, prev)

    nc.compile()
    return nc


# --------------------------------------------------------------------------
# entry point
# --------------------------------------------------------------------------

def kernel(x, vertices, vertex_normals, faces):
    from concourse.bass_utils import run_bass_kernel_spmd

    x = np.asarray(x, np.float32)
    vertices = np.asarray(vertices, np.float32)
    vertex_normals = np.asarray(vertex_normals, np.float32)
    faces = np.asarray(faces, np.int32)

    prep = _host_prep(x, vertices, vertex_normals, faces)
    Vp, Fp = prep['Vp'], prep['Fp']
    n_buckets = prep['xq'].shape[0]
    NB = n_buckets // N_CORES

    key = (Vp, Fp, NB)
    if key not in _compiled_cache:
        _compiled_cache[key] = _build_nc(Vp, Fp, NB)
    nc = _compiled_cache[key]

    in_maps = []
    for c in range(N_CORES):
        sl = slice(c * NB, (c + 1) * NB)
        in_maps.append({
            'xq': np.ascontiguousarray(prep['xq'][sl]),
            'xqc': np.ascontiguousarray(prep['xqc'][sl]),
            'vfeat': np.ascontiguousarray(prep['vfeat'][sl]),
            'vtab': np.ascontiguousarray(prep['vtab'][sl]),
            'wf16': np.ascontiguousarray(prep['wf16'][sl]),
            'wfhl': np.ascontiguousarray(prep['wfhl'][sl]),
        })

    global LAST_RESULT
    outs = None
    for attempt in range(3):
        res = run_bass_kernel_spmd(nc, in_maps, core_ids=list(range(N_CORES)))
        LAST_RESULT = res
        outs = np.concatenate([r['out'].reshape(NB * P, 7)
                               for r in res.results], axis=0)
        # outputs are geometrically bounded (|xc|<~3, |s|<~1, |n|=1); a
        # wildly out-of-range or non-finite result indicates a transient
        # device-state failure -> rerun
        if np.isfinite(outs).all() and np.abs(outs).max() < 1e3:
            break
    assert outs is not None

    N = x.shape[0]
    full = np.zeros((N, 7), np.float32)
    full[prep['perm']] = outs
    xc = np.ascontiguousarray(full[:, 0:3])
    s = np.ascontiguousarray(full[:, 3:4])
    n = np.ascontiguousarray(full[:, 4:7])
    return xc, s, n
